# revision 37
# baseline (speedup 1.0000x reference)
"""Mixtral-style sparse MoE block on 8 Trainium2 NeuronCores.

Strategy: expert-parallel with overflow dealing. The router (tiny: T x H @
H x E) runs on the host as part of input sharding; core e is assigned expert
e and receives up to n_full*512 of its routed tokens, transposed to
feature-major layout. Tokens beyond that capacity ("overflow") are split
into chunks of <= W tokens and dealt to OTHER cores' single W-wide tail
slice, which carries its own copy of the overflow expert's weights. This
keeps the padded per-core token count C = n_full*512 + W close to the
balanced mean instead of the max expert load. The host applies the
renormalized top-2 combine weights and scatter-adds the per-core outputs
back into the full [T, H] output.

Per-core kernel math (C tokens, padded):
    h  = silu(x @ w1) * (x @ w3)    # [C, F] SwiGLU
    y  = h @ w2                     # [C, H]
computed in feature-on-partition layout: activations are [feature, token]
so all three weight matrices are used in their natural [K, M] layout as
matmul stationary operands and the SwiGLU intermediate h lands directly in
the [F-partition, token-free] layout that the down-projection consumes.

All matmul operands are bf16 (fp32 PSUM accumulation), which halves HBM
traffic vs fp32r and enables the PE's fast-weight-load path. Slices are
processed in groups; each weight tile is DMA'd once per group and the
group's h intermediates stay in SBUF between the up- and down-projection
phases, so weight streaming stays far below the PE roofline. Group 0 is a
single slice (its weight demand fits the DMA engines' slow ramp-up) and the
tail slice, which needs a second weight stream, rides first in the LAST
group where DMA capacity is idle and the program still drains on a
full-width slice.
"""

import numpy as np

H = 1024        # hidden dim
F = 3584        # FFN dim
E = 8           # experts == cores
NT = 512        # full token slice (psum bank = 512 fp32)
KH = H // 128   # 8 k-tiles over hidden
MF = F // 128   # 28 m-tiles over ffn
MH = H // 128   # 8 m-tiles over hidden (down-proj output)

_compile_cache = {}
_last_result = None  # BassKernelResults of the most recent run (for profiling)


def _plan(cnt):
    """Choose (n_full, W): per-core capacity n_full*512 main + one W-wide tail.

    Overflow beyond the main capacity is chunked into <= W tokens and dealt
    one chunk per core, so we need sum_e ceil(overflow_e / W) <= E. Searches
    the smallest C = n_full*NT + W; always terminates at the expert-local
    fallback (n_full = ceil(max/NT), W = 0).
    """
    total = sum(cnt)
    max_cnt = max(cnt)
    for n_full in range(max(total // E // NT, 1), -(-max_cnt // NT) + 1):
        cap = n_full * NT
        ov = [c - cap for c in cnt if c > cap]
        if not ov:
            return n_full, 0
        for w in range(32, NT + 1, 16):
            if sum(-(-o // w) for o in ov) <= E:
                return n_full, w
    return -(-max_cnt // NT), 0


def _build(n_full, W):
    """Build + compile the per-core Bass program.

    Slices: one W-wide tail first (overflow tokens, own weight set), then
    n_full 512-wide main slices (the core's own expert).
    """
    import concourse.bass as bass
    import concourse.mybir as mybir
    import concourse.tile as tile
    from concourse import bacc

    slices = ((W,) if W else ()) + (NT,) * n_full
    C = sum(slices)
    f32 = mybir.dt.float32
    bf16 = mybir.dt.bfloat16
    ts = bass.ts

    nc = bacc.Bacc("TRN2", target_bir_lowering=False, debug=False, num_devices=E)

    xT = nc.dram_tensor("xT", [H, C], bf16, kind="ExternalInput").ap()
    w1s = nc.dram_tensor("w1s", [MF, 128, H], bf16, kind="ExternalInput").ap()
    w3s = nc.dram_tensor("w3s", [MF, 128, H], bf16, kind="ExternalInput").ap()
    w2s = nc.dram_tensor("w2s", [MH, 128, F], bf16, kind="ExternalInput").ap()
    if W:
        w1t = nc.dram_tensor("w1t", [MF, 128, H], bf16, kind="ExternalInput").ap()
        w3t = nc.dram_tensor("w3t", [MF, 128, H], bf16, kind="ExternalInput").ap()
        w2t = nc.dram_tensor("w2t", [MH, 128, F], bf16, kind="ExternalInput").ap()
    yT = nc.dram_tensor("yT", [H, C], bf16, kind="ExternalOutput").ap()

    xT_r = xT.rearrange("(k p) t -> k p t", p=128)
    yT_r = yT.rearrange("(m p) t -> m p t", p=128)

    # (offset, width, is_tail) per slice, tail first.
    offs = []
    off = 0
    for i, width in enumerate(slices):
        offs.append((off, width, W > 0 and i == 0))
        off += width

    # Groups share one weight-tile DMA per m-tile. Group 0 is a single slice:
    # its weight demand (0.5 MB / 3.4 us of PE) fits inside the DMA engines'
    # slow ramp-up, so the PE starts without a long fill stall. The tail
    # slice (a second weight set -> double weight DMA) rides in the LAST
    # group, where the DMA queues are otherwise idle; it is computed first
    # within that group so the program still drains on a full slice.
    fulls = offs[1:] if W else offs
    groups = [fulls[:1]]
    rest = fulls[1:]
    groups += [rest[i : i + 2] for i in range(0, len(rest), 2)]
    groups = [g for g in groups if g]
    if W:
        # The tail rides in a MIDDLE group: that group's phase 2 is long
        # enough (3 slices) to absorb the dual w2 weight stream the tail
        # adds, and the program then starts and ends on single-weight-set
        # groups (no DMA pressure at the ramp or the drain).
        mid = len(groups) // 2 if len(groups) > 1 else 0
        groups[mid] = [offs[0]] + groups[mid]
    max_group = max(len(g) for g in groups)

    with tile.TileContext(nc, trace_sim=False) as tc:
        with (
            tc.tile_pool(name="xp", bufs=3) as xp,
            tc.tile_pool(name="w1p", bufs=6) as w1p,
            tc.tile_pool(name="w3p", bufs=6) as w3p,
            tc.tile_pool(name="w2p", bufs=8) as w2p,
            tc.tile_pool(name="hp", bufs=max_group * MF + 4) as hp,
            tc.tile_pool(name="hsp", bufs=3) as hsp,
            tc.tile_pool(name="yp", bufs=3) as yp,
            tc.tile_pool(name="ps1p", bufs=3, space="PSUM") as ps1p,
            tc.tile_pool(name="ps3p", bufs=3, space="PSUM") as ps3p,
            tc.tile_pool(name="psyp", bufs=2, space="PSUM") as psyp,
        ):
            for gi, group in enumerate(groups):
                has_tail = any(t for _, _, t in group)
                has_main = any(not t for _, _, t in group)

                # In the first group, only slice 0's x goes ahead of the first
                # weight tiles (the DMA ramp-up is slow; 1MB of x in front of
                # them would stall the PE on m=1..3). Later groups issue all
                # x up front so it prefetches during the previous group.
                xts = []
                deferred = []
                for si, (off, width, _) in enumerate(group):
                    xt = xp.tile([128, KH, NT], bf16)
                    if gi == 0 and si > 0:
                        deferred.append((xt, off, width))
                    else:
                        for k in range(KH):
                            nc.sync.dma_start(
                                xt[:, k, :width], xT_r[k, :, off : off + width]
                            )
                    xts.append(xt)

                h_tiles = [[] for _ in group]
                w2_pre = {}
                for m in range(MF):
                    w1m = w3m = w1x = w3x = None
                    if has_tail:
                        w1x = w1p.tile([128, H], bf16, tag="w1")
                        nc.sync.dma_start(w1x[:], w1t[m])
                        w3x = w3p.tile([128, H], bf16, tag="w3")
                        nc.sync.dma_start(w3x[:], w3t[m])
                    if has_main:
                        w1m = w1p.tile([128, H], bf16, tag="w1")
                        nc.sync.dma_start(w1m[:], w1s[m])
                        w3m = w3p.tile([128, H], bf16, tag="w3")
                        nc.sync.dma_start(w3m[:], w3s[m])
                    if m == 0:
                        for xt, off, width in deferred:
                            for k in range(KH):
                                nc.sync.dma_start(
                                    xt[:, k, :width], xT_r[k, :, off : off + width]
                                )
                    if m == MF - 3:
                        # prefetch the first down-proj weight tile(s) so the
                        # phase-2 pipeline starts without a DMA wait
                        if has_tail:
                            w2_pre[(True, 0)] = w2p.tile(
                                [128, F], bf16, name="w2pre_t", tag="w2"
                            )
                            nc.sync.dma_start(w2_pre[(True, 0)][:], w2t[0])
                        if has_main:
                            w2_pre[(False, 0)] = w2p.tile(
                                [128, F], bf16, name="w2pre_m", tag="w2"
                            )
                            nc.sync.dma_start(w2_pre[(False, 0)][:], w2s[0])

                    for si, (off, width, is_tail) in enumerate(group):
                        a1 = w1x if is_tail else w1m
                        a3 = w3x if is_tail else w3m
                        xt = xts[si]
                        ps1 = ps1p.tile([128, NT], f32)
                        for k in range(KH):
                            nc.tensor.matmul(
                                ps1[:, :width],
                                a1[:, ts(k, 128)],
                                xt[:, k, :width],
                                start=(k == 0),
                                stop=(k == KH - 1),
                            )
                        ps3 = ps3p.tile([128, NT], f32)
                        for k in range(KH):
                            nc.tensor.matmul(
                                ps3[:, :width],
                                a3[:, ts(k, 128)],
                                xt[:, k, :width],
                                start=(k == 0),
                                stop=(k == KH - 1),
                            )
                        hs = hsp.tile([128, NT], f32)
                        nc.scalar.activation(
                            hs[:, :width], ps1[:, :width],
                            mybir.ActivationFunctionType.Silu,
                        )
                        ht = hp.tile([128, NT], bf16)
                        nc.vector.tensor_mul(
                            ht[:, :width], hs[:, :width], ps3[:, :width]
                        )
                        h_tiles[si].append(ht)

                for mh in range(MH):
                    w2m = w2x = None
                    if has_tail:
                        w2x = w2_pre.get((True, mh))
                        if w2x is None:
                            w2x = w2p.tile([128, F], bf16, tag="w2")
                            nc.sync.dma_start(w2x[:], w2t[mh])
                    if has_main:
                        w2m = w2_pre.get((False, mh))
                        if w2m is None:
                            w2m = w2p.tile([128, F], bf16, tag="w2")
                            nc.sync.dma_start(w2m[:], w2s[mh])
                    for si, (off, width, is_tail) in enumerate(group):
                        a2 = w2x if is_tail else w2m
                        # rotate down-proj accumulators over all 8 PSUM banks
                        # (the phase-1 pools are idle during phase 2)
                        psp, ptag = ((psyp, "psy"), (ps1p, "ps1"), (ps3p, "ps3"))[
                            (mh * len(group) + si) % 3
                        ]
                        psy = psp.tile([128, NT], f32, name="psy", tag=ptag)
                        for kf in range(MF):
                            nc.tensor.matmul(
                                psy[:, :width],
                                a2[:, ts(kf, 128)],
                                h_tiles[si][kf][:, :width],
                                start=(kf == 0),
                                stop=(kf == MF - 1),
                            )
                        yt = yp.tile([128, NT], bf16)
                        nc.vector.tensor_copy(yt[:, :width], psy[:, :width])
                        nc.sync.dma_start(
                            yT_r[mh, :, off : off + width], yt[:, :width]
                        )

    nc.compile()
    return nc


def _route(x, gate_w, gate_b):
    """Host router: top-2 expert ids + renormalized combine weights."""
    logits = x.astype(np.float32) @ gate_w.astype(np.float32).T + gate_b.astype(
        np.float32
    )
    # top-2 by prob == top-2 by logit (softmax is monotonic); stable sort
    # matches jax.lax.top_k's lower-index-first tie-breaking.
    top2 = np.argsort(-logits, axis=-1, kind="stable")[:, :2]
    l2 = np.take_along_axis(logits, top2, axis=1)
    e2 = np.exp(l2 - l2.max(axis=1, keepdims=True))
    wts = e2 / e2.sum(axis=1, keepdims=True)
    return top2, wts.astype(np.float32)


def kernel(x, gate_w, gate_b, w1, w3, w2):
    import ml_dtypes
    from concourse.bass_utils import run_bass_kernel_spmd

    bf16 = ml_dtypes.bfloat16
    x = np.asarray(x, dtype=np.float32)
    T = x.shape[0]
    top2, wts = _route(x, np.asarray(gate_w), np.asarray(gate_b))

    idx_list, scale_list = [], []
    for e in range(E):
        sel = top2 == e                      # [T, 2] bool
        tok = np.nonzero(sel.any(axis=1))[0]
        idx_list.append(tok)
        # each token picks an expert at most once, so this take is unique
        which = sel[tok, 1].astype(np.int64)  # 0 if slot0, 1 if slot1
        scale_list.append(wts[tok, which])

    cnt = [len(i) for i in idx_list]
    n_full, W = _plan(cnt)
    cap = n_full * NT
    C = cap + W

    # Overflow chunks: (expert, token idx, scale), <= W tokens each, one per
    # core's tail slice.
    chunks = []
    for e in range(E):
        for s in range(cap, cnt[e], max(W, 1)):
            chunks.append((e, idx_list[e][s : s + W], scale_list[e][s : s + W]))
    assert len(chunks) <= E, (cnt, n_full, W)
    chunks += [None] * (E - len(chunks))

    nc = _compile_cache.get((n_full, W))
    if nc is None:
        nc = _build(n_full, W)
        _compile_cache[(n_full, W)] = nc

    w1 = np.asarray(w1, dtype=np.float32)
    w3 = np.asarray(w3, dtype=np.float32)
    w2 = np.asarray(w2, dtype=np.float32)

    # W[k*128+p, m*128+c] -> [m, p, k*128+c]: 2KB-contiguous lhsT tiles
    wconv = []
    for e in range(E):
        w1s_e = np.ascontiguousarray(
            w1[e].reshape(KH, 128, MF, 128).transpose(2, 1, 0, 3).reshape(MF, 128, H)
        ).astype(bf16)
        w3s_e = np.ascontiguousarray(
            w3[e].reshape(KH, 128, MF, 128).transpose(2, 1, 0, 3).reshape(MF, 128, H)
        ).astype(bf16)
        w2s_e = np.ascontiguousarray(
            w2[e].reshape(MF, 128, MH, 128).transpose(2, 1, 0, 3).reshape(MH, 128, F)
        ).astype(bf16)
        wconv.append((w1s_e, w3s_e, w2s_e))

    in_maps = []
    for c in range(E):
        tok = idx_list[c][:cap]
        xTe = np.zeros((H, C), bf16)
        xTe[:, W : W + len(tok)] = x[tok].T.astype(bf16)
        im = {"xT": xTe, "w1s": wconv[c][0], "w3s": wconv[c][1], "w2s": wconv[c][2]}
        if W:
            te = chunks[c][0] if chunks[c] else c
            if chunks[c] is not None:
                ttok = chunks[c][1]
                xTe[:, : len(ttok)] = x[ttok].T.astype(bf16)
            im["w1t"], im["w3t"], im["w2t"] = wconv[te]
        in_maps.append(im)

    global _last_result
    res = run_bass_kernel_spmd(nc, in_maps, core_ids=list(range(E)))
    _last_result = res

    out = np.zeros((T, H), np.float32)
    for c in range(E):
        yTe = np.asarray(res.results[c]["yT"]).astype(np.float32)
        tok = idx_list[c][:cap]
        if len(tok):
            out[tok] += yTe[:, W : W + len(tok)].T * scale_list[c][:cap][:, None]
        if W and chunks[c] is not None:
            ttok = chunks[c][1]
            out[ttok] += yTe[:, : len(ttok)].T * chunks[c][2][:, None]
    return out


# revision 38
# speedup vs baseline: 1.0015x; 1.0015x over previous
"""Mixtral-style sparse MoE block on 8 Trainium2 NeuronCores.

Strategy: expert-parallel with overflow dealing. The router (tiny: T x H @
H x E) runs on the host as part of input sharding; core e is assigned expert
e and receives up to n_full*512 of its routed tokens, transposed to
feature-major layout. Tokens beyond that capacity ("overflow") are split
into chunks of <= W tokens and dealt to OTHER cores' single W-wide tail
slice, which carries its own copy of the overflow expert's weights. This
keeps the padded per-core token count C = n_full*512 + W close to the
balanced mean instead of the max expert load. The host applies the
renormalized top-2 combine weights and scatter-adds the per-core outputs
back into the full [T, H] output.

Per-core kernel math (C tokens, padded):
    h  = silu(x @ w1) * (x @ w3)    # [C, F] SwiGLU
    y  = h @ w2                     # [C, H]
computed in feature-on-partition layout: activations are [feature, token]
so all three weight matrices are used in their natural [K, M] layout as
matmul stationary operands and the SwiGLU intermediate h lands directly in
the [F-partition, token-free] layout that the down-projection consumes.

All matmul operands are bf16 (fp32 PSUM accumulation), which halves HBM
traffic vs fp32r and enables the PE's fast-weight-load path. Slices are
processed in groups; each weight tile is DMA'd once per group and the
group's h intermediates stay in SBUF between the up- and down-projection
phases, so weight streaming stays far below the PE roofline. Group 0 is a
single slice (its weight demand fits the DMA engines' slow ramp-up) and the
tail slice, which needs a second weight stream, rides first in the LAST
group where DMA capacity is idle and the program still drains on a
full-width slice.
"""

import numpy as np

H = 1024        # hidden dim
F = 3584        # FFN dim
E = 8           # experts == cores
NT = 512        # full token slice (psum bank = 512 fp32)
KH = H // 128   # 8 k-tiles over hidden
MF = F // 128   # 28 m-tiles over ffn
MH = H // 128   # 8 m-tiles over hidden (down-proj output)

_compile_cache = {}
_last_result = None  # BassKernelResults of the most recent run (for profiling)


def _plan(cnt):
    """Choose (n_full, W): per-core capacity n_full*512 main + one W-wide tail.

    Overflow beyond the main capacity is chunked into <= W tokens and dealt
    one chunk per core, so we need sum_e ceil(overflow_e / W) <= E. Searches
    the smallest C = n_full*NT + W; always terminates at the expert-local
    fallback (n_full = ceil(max/NT), W = 0).
    """
    total = sum(cnt)
    max_cnt = max(cnt)
    for n_full in range(max(total // E // NT, 1), -(-max_cnt // NT) + 1):
        cap = n_full * NT
        ov = [c - cap for c in cnt if c > cap]
        if not ov:
            return n_full, 0
        for w in range(32, NT + 1, 16):
            if sum(-(-o // w) for o in ov) <= E:
                return n_full, w
    return -(-max_cnt // NT), 0


def _build(n_full, W):
    """Build + compile the per-core Bass program.

    Slices: one W-wide tail first (overflow tokens, own weight set), then
    n_full 512-wide main slices (the core's own expert).
    """
    import concourse.bass as bass
    import concourse.mybir as mybir
    import concourse.tile as tile
    from concourse import bacc

    slices = ((W,) if W else ()) + (NT,) * n_full
    C = sum(slices)
    f32 = mybir.dt.float32
    bf16 = mybir.dt.bfloat16
    ts = bass.ts

    nc = bacc.Bacc("TRN2", target_bir_lowering=False, debug=False, num_devices=E)

    xT = nc.dram_tensor("xT", [H, C], bf16, kind="ExternalInput").ap()
    w1s = nc.dram_tensor("w1s", [MF, 128, H], bf16, kind="ExternalInput").ap()
    w3s = nc.dram_tensor("w3s", [MF, 128, H], bf16, kind="ExternalInput").ap()
    w2s = nc.dram_tensor("w2s", [MH, 128, F], bf16, kind="ExternalInput").ap()
    if W:
        w1t = nc.dram_tensor("w1t", [MF, 128, H], bf16, kind="ExternalInput").ap()
        w3t = nc.dram_tensor("w3t", [MF, 128, H], bf16, kind="ExternalInput").ap()
        w2t = nc.dram_tensor("w2t", [MH, 128, F], bf16, kind="ExternalInput").ap()
    yT = nc.dram_tensor("yT", [H, C], bf16, kind="ExternalOutput").ap()

    xT_r = xT.rearrange("(k p) t -> k p t", p=128)
    yT_r = yT.rearrange("(m p) t -> m p t", p=128)

    # (offset, width, is_tail) per slice, tail first.
    offs = []
    off = 0
    for i, width in enumerate(slices):
        offs.append((off, width, W > 0 and i == 0))
        off += width

    # Groups share one weight-tile DMA per m-tile. Group 0 is a single slice:
    # its weight demand (0.5 MB / 3.4 us of PE) fits inside the DMA engines'
    # slow ramp-up, so the PE starts without a long fill stall. The tail
    # slice (a second weight set -> double weight DMA) rides in the LAST
    # group, where the DMA queues are otherwise idle; it is computed first
    # within that group so the program still drains on a full slice.
    fulls = offs[1:] if W else offs
    groups = [fulls[:1]]
    rest = fulls[1:]
    groups += [rest[i : i + 2] for i in range(0, len(rest), 2)]
    groups = [g for g in groups if g]
    if W:
        # The tail rides FIRST in group 0: its LDWEIGHTS-bound chains are a
        # poor fit for the warm full-rate PE but a perfect filler for the
        # kernel-start window, where the PE would otherwise idle on the DMA
        # ramp (and run at HAM half-rate anyway).
        groups[0] = [offs[0]] + groups[0]
    max_group = max(len(g) for g in groups)

    with tile.TileContext(nc, trace_sim=False) as tc:
        with (
            tc.tile_pool(name="xp", bufs=3) as xp,
            tc.tile_pool(name="w1p", bufs=6) as w1p,
            tc.tile_pool(name="w3p", bufs=6) as w3p,
            tc.tile_pool(name="w2p", bufs=8) as w2p,
            tc.tile_pool(name="hp", bufs=max_group * MF + 4) as hp,
            tc.tile_pool(name="hsp", bufs=3) as hsp,
            tc.tile_pool(name="yp", bufs=3) as yp,
            tc.tile_pool(name="ps1p", bufs=3, space="PSUM") as ps1p,
            tc.tile_pool(name="ps3p", bufs=3, space="PSUM") as ps3p,
            tc.tile_pool(name="psyp", bufs=2, space="PSUM") as psyp,
        ):
            for gi, group in enumerate(groups):
                has_tail = any(t for _, _, t in group)
                has_main = any(not t for _, _, t in group)

                # In the first group, only slice 0's x goes ahead of the first
                # weight tiles (the DMA ramp-up is slow; 1MB of x in front of
                # them would stall the PE on m=1..3). Later groups issue all
                # x up front so it prefetches during the previous group.
                xts = []
                deferred = []
                for si, (off, width, _) in enumerate(group):
                    xt = xp.tile([128, KH, NT], bf16)
                    if gi == 0 and si > 0:
                        deferred.append((xt, off, width))
                    else:
                        for k in range(KH):
                            nc.sync.dma_start(
                                xt[:, k, :width], xT_r[k, :, off : off + width]
                            )
                    xts.append(xt)

                h_tiles = [[] for _ in group]
                w2_pre = {}
                for m in range(MF):
                    w1m = w3m = w1x = w3x = None
                    if has_tail:
                        w1x = w1p.tile([128, H], bf16, tag="w1")
                        nc.sync.dma_start(w1x[:], w1t[m])
                        w3x = w3p.tile([128, H], bf16, tag="w3")
                        nc.sync.dma_start(w3x[:], w3t[m])
                    if has_main:
                        w1m = w1p.tile([128, H], bf16, tag="w1")
                        nc.sync.dma_start(w1m[:], w1s[m])
                        w3m = w3p.tile([128, H], bf16, tag="w3")
                        nc.sync.dma_start(w3m[:], w3s[m])
                    if m == 0:
                        for xt, off, width in deferred:
                            for k in range(KH):
                                nc.sync.dma_start(
                                    xt[:, k, :width], xT_r[k, :, off : off + width]
                                )
                    if m == MF - 3:
                        # prefetch the first down-proj weight tile(s) so the
                        # phase-2 pipeline starts without a DMA wait
                        if has_tail:
                            w2_pre[(True, 0)] = w2p.tile(
                                [128, F], bf16, name="w2pre_t", tag="w2"
                            )
                            nc.sync.dma_start(w2_pre[(True, 0)][:], w2t[0])
                        if has_main:
                            w2_pre[(False, 0)] = w2p.tile(
                                [128, F], bf16, name="w2pre_m", tag="w2"
                            )
                            nc.sync.dma_start(w2_pre[(False, 0)][:], w2s[0])

                    for si, (off, width, is_tail) in enumerate(group):
                        a1 = w1x if is_tail else w1m
                        a3 = w3x if is_tail else w3m
                        xt = xts[si]
                        ps1 = ps1p.tile([128, NT], f32)
                        for k in range(KH):
                            nc.tensor.matmul(
                                ps1[:, :width],
                                a1[:, ts(k, 128)],
                                xt[:, k, :width],
                                start=(k == 0),
                                stop=(k == KH - 1),
                            )
                        ps3 = ps3p.tile([128, NT], f32)
                        for k in range(KH):
                            nc.tensor.matmul(
                                ps3[:, :width],
                                a3[:, ts(k, 128)],
                                xt[:, k, :width],
                                start=(k == 0),
                                stop=(k == KH - 1),
                            )
                        hs = hsp.tile([128, NT], f32)
                        nc.scalar.activation(
                            hs[:, :width], ps1[:, :width],
                            mybir.ActivationFunctionType.Silu,
                        )
                        ht = hp.tile([128, NT], bf16)
                        nc.vector.tensor_mul(
                            ht[:, :width], hs[:, :width], ps3[:, :width]
                        )
                        h_tiles[si].append(ht)

                for mh in range(MH):
                    w2m = w2x = None
                    if has_tail:
                        w2x = w2_pre.get((True, mh))
                        if w2x is None:
                            w2x = w2p.tile([128, F], bf16, tag="w2")
                            nc.sync.dma_start(w2x[:], w2t[mh])
                    if has_main:
                        w2m = w2_pre.get((False, mh))
                        if w2m is None:
                            w2m = w2p.tile([128, F], bf16, tag="w2")
                            nc.sync.dma_start(w2m[:], w2s[mh])
                    for si, (off, width, is_tail) in enumerate(group):
                        a2 = w2x if is_tail else w2m
                        # rotate down-proj accumulators over all 8 PSUM banks
                        # (the phase-1 pools are idle during phase 2)
                        psp, ptag = ((psyp, "psy"), (ps1p, "ps1"), (ps3p, "ps3"))[
                            (mh * len(group) + si) % 3
                        ]
                        psy = psp.tile([128, NT], f32, name="psy", tag=ptag)
                        for kf in range(MF):
                            nc.tensor.matmul(
                                psy[:, :width],
                                a2[:, ts(kf, 128)],
                                h_tiles[si][kf][:, :width],
                                start=(kf == 0),
                                stop=(kf == MF - 1),
                            )
                        yt = yp.tile([128, NT], bf16)
                        nc.vector.tensor_copy(yt[:, :width], psy[:, :width])
                        nc.sync.dma_start(
                            yT_r[mh, :, off : off + width], yt[:, :width]
                        )

    nc.compile()
    return nc


def _route(x, gate_w, gate_b):
    """Host router: top-2 expert ids + renormalized combine weights."""
    logits = x.astype(np.float32) @ gate_w.astype(np.float32).T + gate_b.astype(
        np.float32
    )
    # top-2 by prob == top-2 by logit (softmax is monotonic); stable sort
    # matches jax.lax.top_k's lower-index-first tie-breaking.
    top2 = np.argsort(-logits, axis=-1, kind="stable")[:, :2]
    l2 = np.take_along_axis(logits, top2, axis=1)
    e2 = np.exp(l2 - l2.max(axis=1, keepdims=True))
    wts = e2 / e2.sum(axis=1, keepdims=True)
    return top2, wts.astype(np.float32)


def kernel(x, gate_w, gate_b, w1, w3, w2):
    import ml_dtypes
    from concourse.bass_utils import run_bass_kernel_spmd

    bf16 = ml_dtypes.bfloat16
    x = np.asarray(x, dtype=np.float32)
    T = x.shape[0]
    top2, wts = _route(x, np.asarray(gate_w), np.asarray(gate_b))

    idx_list, scale_list = [], []
    for e in range(E):
        sel = top2 == e                      # [T, 2] bool
        tok = np.nonzero(sel.any(axis=1))[0]
        idx_list.append(tok)
        # each token picks an expert at most once, so this take is unique
        which = sel[tok, 1].astype(np.int64)  # 0 if slot0, 1 if slot1
        scale_list.append(wts[tok, which])

    cnt = [len(i) for i in idx_list]
    n_full, W = _plan(cnt)
    cap = n_full * NT
    C = cap + W

    # Overflow chunks: (expert, token idx, scale), <= W tokens each, one per
    # core's tail slice.
    chunks = []
    for e in range(E):
        for s in range(cap, cnt[e], max(W, 1)):
            chunks.append((e, idx_list[e][s : s + W], scale_list[e][s : s + W]))
    assert len(chunks) <= E, (cnt, n_full, W)
    chunks += [None] * (E - len(chunks))

    nc = _compile_cache.get((n_full, W))
    if nc is None:
        nc = _build(n_full, W)
        _compile_cache[(n_full, W)] = nc

    w1 = np.asarray(w1, dtype=np.float32)
    w3 = np.asarray(w3, dtype=np.float32)
    w2 = np.asarray(w2, dtype=np.float32)

    # W[k*128+p, m*128+c] -> [m, p, k*128+c]: 2KB-contiguous lhsT tiles
    wconv = []
    for e in range(E):
        w1s_e = np.ascontiguousarray(
            w1[e].reshape(KH, 128, MF, 128).transpose(2, 1, 0, 3).reshape(MF, 128, H)
        ).astype(bf16)
        w3s_e = np.ascontiguousarray(
            w3[e].reshape(KH, 128, MF, 128).transpose(2, 1, 0, 3).reshape(MF, 128, H)
        ).astype(bf16)
        w2s_e = np.ascontiguousarray(
            w2[e].reshape(MF, 128, MH, 128).transpose(2, 1, 0, 3).reshape(MH, 128, F)
        ).astype(bf16)
        wconv.append((w1s_e, w3s_e, w2s_e))

    in_maps = []
    for c in range(E):
        tok = idx_list[c][:cap]
        xTe = np.zeros((H, C), bf16)
        xTe[:, W : W + len(tok)] = x[tok].T.astype(bf16)
        im = {"xT": xTe, "w1s": wconv[c][0], "w3s": wconv[c][1], "w2s": wconv[c][2]}
        if W:
            te = chunks[c][0] if chunks[c] else c
            if chunks[c] is not None:
                ttok = chunks[c][1]
                xTe[:, : len(ttok)] = x[ttok].T.astype(bf16)
            im["w1t"], im["w3t"], im["w2t"] = wconv[te]
        in_maps.append(im)

    global _last_result
    res = run_bass_kernel_spmd(nc, in_maps, core_ids=list(range(E)))
    _last_result = res

    out = np.zeros((T, H), np.float32)
    for c in range(E):
        yTe = np.asarray(res.results[c]["yT"]).astype(np.float32)
        tok = idx_list[c][:cap]
        if len(tok):
            out[tok] += yTe[:, W : W + len(tok)].T * scale_list[c][:cap][:, None]
        if W and chunks[c] is not None:
            ttok = chunks[c][1]
            out[ttok] += yTe[:, : len(ttok)].T * chunks[c][2][:, None]
    return out


# revision 39
# speedup vs baseline: 1.0050x; 1.0035x over previous
"""Mixtral-style sparse MoE block on 8 Trainium2 NeuronCores.

Strategy: expert-parallel with overflow dealing. The router (tiny: T x H @
H x E) runs on the host as part of input sharding; core e is assigned expert
e and receives up to n_full*512 of its routed tokens, transposed to
feature-major layout. Tokens beyond that capacity ("overflow") are split
into chunks of <= W tokens and dealt to OTHER cores' single W-wide tail
slice, which carries its own copy of the overflow expert's weights. This
keeps the padded per-core token count C = n_full*512 + W close to the
balanced mean instead of the max expert load. The host applies the
renormalized top-2 combine weights and scatter-adds the per-core outputs
back into the full [T, H] output.

Per-core kernel math (C tokens, padded):
    h  = silu(x @ w1) * (x @ w3)    # [C, F] SwiGLU
    y  = h @ w2                     # [C, H]
computed in feature-on-partition layout: activations are [feature, token]
so all three weight matrices are used in their natural [K, M] layout as
matmul stationary operands and the SwiGLU intermediate h lands directly in
the [F-partition, token-free] layout that the down-projection consumes.

All matmul operands are bf16 (fp32 PSUM accumulation), which halves HBM
traffic vs fp32r and enables the PE's fast-weight-load path. Slices are
processed in groups; each weight tile is DMA'd once per group and the
group's h intermediates stay in SBUF between the up- and down-projection
phases, so weight streaming stays far below the PE roofline. Group 0 is a
single slice (its weight demand fits the DMA engines' slow ramp-up) and the
tail slice, which needs a second weight stream, rides first in the LAST
group where DMA capacity is idle and the program still drains on a
full-width slice.
"""

import numpy as np

H = 1024        # hidden dim
F = 3584        # FFN dim
E = 8           # experts == cores
NT = 512        # full token slice (psum bank = 512 fp32)
KH = H // 128   # 8 k-tiles over hidden
MF = F // 128   # 28 m-tiles over ffn
MH = H // 128   # 8 m-tiles over hidden (down-proj output)

_compile_cache = {}
_last_result = None  # BassKernelResults of the most recent run (for profiling)


def _plan(cnt):
    """Choose (n_full, W): per-core capacity n_full*512 main + one W-wide tail.

    Overflow beyond the main capacity is chunked into <= W tokens and dealt
    one chunk per core, so we need sum_e ceil(overflow_e / W) <= E. Searches
    the smallest C = n_full*NT + W; always terminates at the expert-local
    fallback (n_full = ceil(max/NT), W = 0).
    """
    total = sum(cnt)
    max_cnt = max(cnt)
    for n_full in range(max(total // E // NT, 1), -(-max_cnt // NT) + 1):
        cap = n_full * NT
        ov = [c - cap for c in cnt if c > cap]
        if not ov:
            return n_full, 0
        for w in range(32, NT + 1, 16):
            if sum(-(-o // w) for o in ov) <= E:
                return n_full, w
    return -(-max_cnt // NT), 0


def _build(n_full, W):
    """Build + compile the per-core Bass program.

    Slices: one W-wide tail first (overflow tokens, own weight set), then
    n_full 512-wide main slices (the core's own expert).
    """
    import concourse.bass as bass
    import concourse.mybir as mybir
    import concourse.tile as tile
    from concourse import bacc

    slices = ((W,) if W else ()) + (NT,) * n_full
    C = sum(slices)
    f32 = mybir.dt.float32
    bf16 = mybir.dt.bfloat16
    ts = bass.ts

    nc = bacc.Bacc("TRN2", target_bir_lowering=False, debug=False, num_devices=E)

    xT = nc.dram_tensor("xT", [H, C], bf16, kind="ExternalInput").ap()
    w1s = nc.dram_tensor("w1s", [MF, 128, H], bf16, kind="ExternalInput").ap()
    w3s = nc.dram_tensor("w3s", [MF, 128, H], bf16, kind="ExternalInput").ap()
    w2s = nc.dram_tensor("w2s", [MH, 128, F], bf16, kind="ExternalInput").ap()
    if W:
        w1t = nc.dram_tensor("w1t", [MF, 128, H], bf16, kind="ExternalInput").ap()
        w3t = nc.dram_tensor("w3t", [MF, 128, H], bf16, kind="ExternalInput").ap()
        w2t = nc.dram_tensor("w2t", [MH, 128, F], bf16, kind="ExternalInput").ap()
    yT = nc.dram_tensor("yT", [H, C], bf16, kind="ExternalOutput").ap()

    xT_r = xT.rearrange("(k p) t -> k p t", p=128)
    yT_r = yT.rearrange("(m p) t -> m p t", p=128)

    # (offset, width, is_tail) per slice, tail first.
    offs = []
    off = 0
    for i, width in enumerate(slices):
        offs.append((off, width, W > 0 and i == 0))
        off += width

    # Groups share one weight-tile DMA per m-tile. Group 0 is a single slice:
    # its weight demand (0.5 MB / 3.4 us of PE) fits inside the DMA engines'
    # slow ramp-up, so the PE starts without a long fill stall. The tail
    # slice (a second weight set -> double weight DMA) rides in the LAST
    # group, where the DMA queues are otherwise idle; it is computed first
    # within that group so the program still drains on a full slice.
    fulls = offs[1:] if W else offs
    groups = [fulls[:1]]
    rest = fulls[1:]
    groups += [rest[i : i + 2] for i in range(0, len(rest), 2)]
    groups = [g for g in groups if g]
    if W:
        # The tail rides in a MIDDLE group: that group's phase 2 is long
        # enough (3 slices) to absorb the dual w2 weight stream the tail
        # adds, and the program then starts and ends on single-weight-set
        # groups (no DMA pressure at the ramp or the drain).
        mid = len(groups) // 2 if len(groups) > 1 else 0
        groups[mid] = [offs[0]] + groups[mid]
    max_group = max(len(g) for g in groups)

    with tile.TileContext(nc, trace_sim=False) as tc:
        with (
            tc.tile_pool(name="xp", bufs=3) as xp,
            tc.tile_pool(name="w1p", bufs=6) as w1p,
            tc.tile_pool(name="w3p", bufs=6) as w3p,
            tc.tile_pool(name="w2p", bufs=8) as w2p,
            tc.tile_pool(name="hp", bufs=max_group * MF + 4) as hp,
            tc.tile_pool(name="hsp", bufs=3) as hsp,
            tc.tile_pool(name="yp", bufs=3) as yp,
            tc.tile_pool(name="ps1p", bufs=3, space="PSUM") as ps1p,
            tc.tile_pool(name="ps3p", bufs=3, space="PSUM") as ps3p,
            tc.tile_pool(name="psyp", bufs=2, space="PSUM") as psyp,
        ):
            for gi, group in enumerate(groups):
                has_tail = any(t for _, _, t in group)
                has_main = any(not t for _, _, t in group)

                # In the first group, only slice 0's x goes ahead of the first
                # weight tiles (the DMA ramp-up is slow; 1MB of x in front of
                # them would stall the PE on m=1..3). Later groups issue all
                # x up front so it prefetches during the previous group.
                xts = []
                deferred = []
                for si, (off, width, _) in enumerate(group):
                    xt = xp.tile([128, KH, NT], bf16)
                    if gi == 0 and si > 0:
                        deferred.append((xt, off, width))
                    else:
                        for k in range(KH):
                            nc.sync.dma_start(
                                xt[:, k, :width], xT_r[k, :, off : off + width]
                            )
                    xts.append(xt)

                h_tiles = [[] for _ in group]
                w2_pre = {}
                for m in range(MF):
                    w1m = w3m = w1x = w3x = None
                    if has_tail:
                        w1x = w1p.tile([128, H], bf16, tag="w1")
                        nc.sync.dma_start(w1x[:], w1t[m])
                        w3x = w3p.tile([128, H], bf16, tag="w3")
                        nc.sync.dma_start(w3x[:], w3t[m])
                    if has_main:
                        w1m = w1p.tile([128, H], bf16, tag="w1")
                        nc.sync.dma_start(w1m[:], w1s[m])
                        w3m = w3p.tile([128, H], bf16, tag="w3")
                        nc.sync.dma_start(w3m[:], w3s[m])
                    if m == 0:
                        for xt, off, width in deferred:
                            for k in range(KH):
                                nc.sync.dma_start(
                                    xt[:, k, :width], xT_r[k, :, off : off + width]
                                )
                    if m == MF - 3:
                        # prefetch the first down-proj weight tile(s) so the
                        # phase-2 pipeline starts without a DMA wait
                        if has_tail:
                            w2_pre[(True, 0)] = w2p.tile(
                                [128, F], bf16, name="w2pre_t", tag="w2"
                            )
                            nc.sync.dma_start(w2_pre[(True, 0)][:], w2t[0])
                        if has_main:
                            w2_pre[(False, 0)] = w2p.tile(
                                [128, F], bf16, name="w2pre_m", tag="w2"
                            )
                            nc.sync.dma_start(w2_pre[(False, 0)][:], w2s[0])

                    for si, (off, width, is_tail) in enumerate(group):
                        a1 = w1x if is_tail else w1m
                        a3 = w3x if is_tail else w3m
                        xt = xts[si]
                        ps1 = ps1p.tile([128, NT], f32)
                        for k in range(KH):
                            nc.tensor.matmul(
                                ps1[:, :width],
                                a1[:, ts(k, 128)],
                                xt[:, k, :width],
                                start=(k == 0),
                                stop=(k == KH - 1),
                            )
                        ps3 = ps3p.tile([128, NT], f32)
                        for k in range(KH):
                            nc.tensor.matmul(
                                ps3[:, :width],
                                a3[:, ts(k, 128)],
                                xt[:, k, :width],
                                start=(k == 0),
                                stop=(k == KH - 1),
                            )
                        hs = hsp.tile([128, NT], f32)
                        nc.scalar.activation(
                            hs[:, :width], ps1[:, :width],
                            mybir.ActivationFunctionType.Silu,
                        )
                        ht = hp.tile([128, NT], bf16)
                        nc.vector.tensor_mul(
                            ht[:, :width], hs[:, :width], ps3[:, :width]
                        )
                        h_tiles[si].append(ht)

                for mh in range(MH):
                    w2m = w2x = None
                    if has_tail:
                        w2x = w2_pre.get((True, mh))
                        if w2x is None:
                            w2x = w2p.tile([128, F], bf16, tag="w2")
                            nc.sync.dma_start(w2x[:], w2t[mh])
                    if has_main:
                        w2m = w2_pre.get((False, mh))
                        if w2m is None:
                            w2m = w2p.tile([128, F], bf16, tag="w2")
                            nc.sync.dma_start(w2m[:], w2s[mh])
                    for si, (off, width, is_tail) in enumerate(group):
                        a2 = w2x if is_tail else w2m
                        # rotate down-proj accumulators over all 8 PSUM banks
                        # (the phase-1 pools are idle during phase 2)
                        psp, ptag = ((psyp, "psy"), (ps1p, "ps1"), (ps3p, "ps3"))[
                            (mh * len(group) + si) % 3
                        ]
                        psy = psp.tile([128, NT], f32, name="psy", tag=ptag)
                        for kf in range(MF):
                            nc.tensor.matmul(
                                psy[:, :width],
                                a2[:, ts(kf, 128)],
                                h_tiles[si][kf][:, :width],
                                start=(kf == 0),
                                stop=(kf == MF - 1),
                            )
                        yt = yp.tile([128, NT], bf16)
                        nc.vector.tensor_copy(yt[:, :width], psy[:, :width])
                        nc.sync.dma_start(
                            yT_r[mh, :, off : off + width], yt[:, :width]
                        )

    nc.compile()
    return nc


def _route(x, gate_w, gate_b):
    """Host router: top-2 expert ids + renormalized combine weights."""
    logits = x.astype(np.float32) @ gate_w.astype(np.float32).T + gate_b.astype(
        np.float32
    )
    # top-2 by prob == top-2 by logit (softmax is monotonic); stable sort
    # matches jax.lax.top_k's lower-index-first tie-breaking.
    top2 = np.argsort(-logits, axis=-1, kind="stable")[:, :2]
    l2 = np.take_along_axis(logits, top2, axis=1)
    e2 = np.exp(l2 - l2.max(axis=1, keepdims=True))
    wts = e2 / e2.sum(axis=1, keepdims=True)
    return top2, wts.astype(np.float32)


def kernel(x, gate_w, gate_b, w1, w3, w2):
    import ml_dtypes
    from concourse.bass_utils import run_bass_kernel_spmd

    bf16 = ml_dtypes.bfloat16
    x = np.asarray(x, dtype=np.float32)
    T = x.shape[0]
    top2, wts = _route(x, np.asarray(gate_w), np.asarray(gate_b))

    idx_list, scale_list = [], []
    for e in range(E):
        sel = top2 == e                      # [T, 2] bool
        tok = np.nonzero(sel.any(axis=1))[0]
        idx_list.append(tok)
        # each token picks an expert at most once, so this take is unique
        which = sel[tok, 1].astype(np.int64)  # 0 if slot0, 1 if slot1
        scale_list.append(wts[tok, which])

    cnt = [len(i) for i in idx_list]
    n_full, W = _plan(cnt)
    cap = n_full * NT
    C = cap + W

    # Overflow chunks: (expert, token idx, scale), <= W tokens each, one per
    # core's tail slice.
    chunks = []
    for e in range(E):
        for s in range(cap, cnt[e], max(W, 1)):
            chunks.append((e, idx_list[e][s : s + W], scale_list[e][s : s + W]))
    assert len(chunks) <= E, (cnt, n_full, W)
    chunks += [None] * (E - len(chunks))

    nc = _compile_cache.get((n_full, W))
    if nc is None:
        nc = _build(n_full, W)
        _compile_cache[(n_full, W)] = nc

    w1 = np.asarray(w1, dtype=np.float32)
    w3 = np.asarray(w3, dtype=np.float32)
    w2 = np.asarray(w2, dtype=np.float32)

    # W[k*128+p, m*128+c] -> [m, p, k*128+c]: 2KB-contiguous lhsT tiles
    wconv = []
    for e in range(E):
        w1s_e = np.ascontiguousarray(
            w1[e].reshape(KH, 128, MF, 128).transpose(2, 1, 0, 3).reshape(MF, 128, H)
        ).astype(bf16)
        w3s_e = np.ascontiguousarray(
            w3[e].reshape(KH, 128, MF, 128).transpose(2, 1, 0, 3).reshape(MF, 128, H)
        ).astype(bf16)
        w2s_e = np.ascontiguousarray(
            w2[e].reshape(MF, 128, MH, 128).transpose(2, 1, 0, 3).reshape(MH, 128, F)
        ).astype(bf16)
        wconv.append((w1s_e, w3s_e, w2s_e))

    in_maps = []
    for c in range(E):
        tok = idx_list[c][:cap]
        xTe = np.zeros((H, C), bf16)
        xTe[:, W : W + len(tok)] = x[tok].T.astype(bf16)
        im = {"xT": xTe, "w1s": wconv[c][0], "w3s": wconv[c][1], "w2s": wconv[c][2]}
        if W:
            te = chunks[c][0] if chunks[c] else c
            if chunks[c] is not None:
                ttok = chunks[c][1]
                xTe[:, : len(ttok)] = x[ttok].T.astype(bf16)
            im["w1t"], im["w3t"], im["w2t"] = wconv[te]
        in_maps.append(im)

    global _last_result
    res = run_bass_kernel_spmd(nc, in_maps, core_ids=list(range(E)))
    _last_result = res

    out = np.zeros((T, H), np.float32)
    for c in range(E):
        yTe = np.asarray(res.results[c]["yT"]).astype(np.float32)
        tok = idx_list[c][:cap]
        if len(tok):
            out[tok] += yTe[:, W : W + len(tok)].T * scale_list[c][:cap][:, None]
        if W and chunks[c] is not None:
            ttok = chunks[c][1]
            out[ttok] += yTe[:, : len(ttok)].T * chunks[c][2][:, None]
    return out
